# revision 12
# baseline (speedup 1.0000x reference)
"""EnsembleFC (E=16 MLPs, 512->512->512->1, relu) on 8 TRN2 NeuronCores.

Strategy (expert parallel): each core owns E/8 = 2 ensemble members' weights
and computes their [B] output column; x is replicated. All activations stay
in "feature-major" (transposed) layout so no on-device transposes are needed:

    h1^T = relu(W1^T @ x^T + b1)      [H, B]
    h2^T = relu(W2^T @ h1^T + b2)     [H, B]
    out^T = W3^T @ h2^T               [1, B]   (b3 added on host)

Matmuls run in float32r (TRN2 reduced-precision fp32 PE mode, 1 cycle/row —
4x faster than plain fp32, ~20x more accurate than bf16; measured scaled
error ~1.5e-4 per 128-deep contraction).

Raw Bass (no Tile framework): this container's walrus rejects instructions
with >1-2 sync waits, which Tile's auto-generated drains exceed. Explicit
per-engine programs with standalone waits keep every instruction at one wait.
"""
import numpy as np

E, D, H, B = 16, 512, 512, 8192
N_CORES = 8
MPC = E // N_CORES          # members per core
KT = D // 128               # k-tiles per 512 contraction
MT = H // 128               # m-tiles per 512 output dim
CH = 512                    # batch columns per chunk (one psum bank)
NCH = B // CH               # chunks
XBUF = 3                    # x chunk double/triple buffering

_CACHE = {}


def _build():
    import concourse.bass as bass
    from concourse import mybir

    f32 = mybir.dt.float32
    f32r = mybir.dt.float32r

    nc = bass.Bass("TRN2", target_bir_lowering=False, debug=False,
                   num_devices=N_CORES)

    xT = nc.dram_tensor("xT", [D, B], f32r, kind="ExternalInput").ap()
    w1 = nc.dram_tensor("w1", [MPC, D, H], f32r, kind="ExternalInput").ap()
    w2 = nc.dram_tensor("w2", [MPC, H, H], f32r, kind="ExternalInput").ap()
    # host-side pre-arranged: w3[p, m, kt], b1/b2[p, m, mt]
    w3 = nc.dram_tensor("w3", [128, MPC, KT], f32r, kind="ExternalInput").ap()
    b1 = nc.dram_tensor("b1", [128, MPC, MT], f32, kind="ExternalInput").ap()
    b2 = nc.dram_tensor("b2", [128, MPC, MT], f32, kind="ExternalInput").ap()
    out = nc.dram_tensor("out", [MPC, B], f32, kind="ExternalOutput").ap()

    w1s = [nc.alloc_sbuf_tensor(f"w1s{m}", [128, KT, H], f32r).ap()
           for m in range(MPC)]
    w2s = [nc.alloc_sbuf_tensor(f"w2s{m}", [128, KT, H], f32r).ap()
           for m in range(MPC)]
    w3s = nc.alloc_sbuf_tensor("w3s", [128, MPC, KT], f32r).ap()
    b1s = nc.alloc_sbuf_tensor("b1s", [128, MPC, MT], f32).ap()
    b2s = nc.alloc_sbuf_tensor("b2s", [128, MPC, MT], f32).ap()
    xs = nc.alloc_sbuf_tensor("xs", [128, XBUF, KT, CH], f32r).ap()
    h1 = nc.alloc_sbuf_tensor("h1", [128, MPC, KT, CH], f32r).ap()
    h2 = nc.alloc_sbuf_tensor("h2", [128, MPC, KT, CH], f32r).ap()
    # per-member output staging, both at partition 0 (L3 psum lands in a
    # different bank per member, always at partition 0)
    osb = [nc.alloc_sbuf_tensor(f"osb{m}", [1, NCH, CH], f32).ap()
           for m in range(MPC)]

    psA = nc.alloc_psum_tensor("psA", [128, MT, CH], f32).ap()   # L1
    psB = nc.alloc_psum_tensor("psB", [128, MT, CH], f32).ap()   # L2 (+L3 in bank 0)

    xT_r = xT.rearrange("(kt p) b -> p kt b", p=128)

    # semaphore tick bookkeeping (absolute counts)
    def mm_l1(c, m, mt):            # after L1 group (c, m, mt)
        return 18 * c + 9 * m + mt + 1

    def mm_l2(c, m, mt):
        return 18 * c + 9 * m + 4 + mt + 1

    def mm_l3(c, m):
        return 18 * c + 9 * m + 9

    def act_l1(c, m):               # after the 4 L1 relus of (c, m)
        return 18 * c + 9 * m + 4

    def act_l2(c, m):
        return 18 * c + 9 * m + 8

    def act_out(c, m):
        return 18 * c + 9 * m + 9

    N_W_DMAS = 2 * MPC + 3          # w1s*, w2s*, w3s, b1s, b2s

    with (
        nc.Block() as block,
        nc.semaphore("w_sem") as w_sem,
        nc.semaphore("mm_sem") as mm_sem,
        nc.semaphore("act_sem") as act_sem,
        nc.semaphore("d_sem") as d_sem,
    ):
        # per-slot x semaphores: DMA queue completions are unordered across
        # chunks, so a single cumulative counter would be racy
        x_sems = [nc.alloc_semaphore(f"x_sem{s}") for s in range(XBUF)]
        @block.sync
        def _(sync: bass.BassEngine):
            for m in range(MPC):
                sync.dma_start(
                    out=w1s[m], in_=w1[m].rearrange("(kt p) m2 -> p kt m2", p=128)
                ).then_inc(w_sem, 16)
                sync.dma_start(
                    out=w2s[m], in_=w2[m].rearrange("(kt p) m2 -> p kt m2", p=128)
                ).then_inc(w_sem, 16)
            sync.dma_start(out=w3s, in_=w3).then_inc(w_sem, 16)
            sync.dma_start(out=b1s, in_=b1).then_inc(w_sem, 16)
            sync.dma_start(out=b2s, in_=b2).then_inc(w_sem, 16)

            for c in range(NCH):
                if c >= XBUF:
                    # x slot free once L1 of chunk c-XBUF fully consumed it
                    sync.wait_ge(mm_sem, mm_l1(c - XBUF, MPC - 1, MT - 1))
                for kt in range(KT):
                    sync.dma_start(
                        out=xs[:, c % XBUF, kt, :],
                        in_=xT_r[:, kt, c * CH:(c + 1) * CH],
                    ).then_inc(x_sems[c % XBUF], 16)

            out_r = out.rearrange("m (nch ch) -> m nch ch", ch=CH)
            sync.wait_ge(act_sem, act_out(NCH - 1, MPC - 1))
            for m in range(MPC):
                sync.dma_start(out=out_r[m:m + 1], in_=osb[m]).then_inc(d_sem, 16)
            sync.wait_ge(d_sem, 16 * MPC)

        @block.tensor
        def _(tensor: bass.BassEngine):
            tensor.wait_ge(w_sem, 16 * N_W_DMAS)
            for c in range(NCH):
                tensor.wait_ge(x_sems[c % XBUF], 64 * (c // XBUF + 1))
                if c >= 1:
                    # psA banks drained by chunk c-1's last L1 relu set
                    tensor.wait_ge(act_sem, act_l1(c - 1, MPC - 1))
                for m in range(MPC):
                    # L1: psA[mt] += W1[kt,mt]^T @ xT[kt]
                    for mt in range(MT):
                        for kt in range(KT):
                            ins = tensor.matmul(
                                psA[:, mt, :],
                                w1s[m][:, kt, mt * 128:(mt + 1) * 128],
                                xs[:, c % XBUF, kt, :],
                                start=(kt == 0), stop=(kt == KT - 1),
                            )
                        ins.then_inc(mm_sem, 1)
                    # L2 needs all 4 h1 tiles of this member
                    tensor.wait_ge(act_sem, act_l1(c, m))
                    for mt in range(MT):
                        for kt in range(KT):
                            ins = tensor.matmul(
                                psB[:, mt, :],
                                w2s[m][:, kt, mt * 128:(mt + 1) * 128],
                                h1[:, m, kt, :],
                                start=(kt == 0), stop=(kt == KT - 1),
                            )
                        ins.then_inc(mm_sem, 1)
                    # L3 into psB bank m, partition 0
                    tensor.wait_ge(act_sem, act_l2(c, m))
                    for kt in range(KT):
                        ins = tensor.matmul(
                            psB[0:1, m, :],
                            w3s[:, m, kt:kt + 1],
                            h2[:, m, kt, :],
                            start=(kt == 0), stop=(kt == KT - 1),
                        )
                    ins.then_inc(mm_sem, 1)

        @block.scalar
        def _(scalar: bass.BassEngine):
            Relu = bass.mybir.ActivationFunctionType.Relu
            for c in range(NCH):
                for m in range(MPC):
                    for mt in range(MT):
                        scalar.wait_ge(mm_sem, mm_l1(c, m, mt))
                        scalar.activation(
                            h1[:, m, mt, :], psA[:, mt, :], Relu,
                            bias=b1s[:, m, mt:mt + 1],
                        ).then_inc(act_sem, 1)
                    for mt in range(MT):
                        scalar.wait_ge(mm_sem, mm_l2(c, m, mt))
                        scalar.activation(
                            h2[:, m, mt, :], psB[:, mt, :], Relu,
                            bias=b2s[:, m, mt:mt + 1],
                        ).then_inc(act_sem, 1)
                    scalar.wait_ge(mm_sem, mm_l3(c, m))
                    scalar.copy(
                        osb[m][0:1, c, :],
                        psB[0:1, m, :],
                    ).then_inc(act_sem, 1)

    return nc


def get_nc():
    if "nc" not in _CACHE:
        _CACHE["nc"] = _build()
    return _CACHE["nc"]


def kernel(x, W1, b1, W2, b2, W3, b3):
    from concourse.bass_utils import run_bass_kernel_spmd

    nc = get_nc()
    xT = np.ascontiguousarray(np.asarray(x, dtype=np.float32).T)
    W1 = np.asarray(W1, dtype=np.float32)
    W2 = np.asarray(W2, dtype=np.float32)
    W3 = np.asarray(W3, dtype=np.float32)
    b1 = np.asarray(b1, dtype=np.float32)
    b2 = np.asarray(b2, dtype=np.float32)
    b3 = np.asarray(b3, dtype=np.float32)

    def feat_major(v):
        # [MPC, H] -> [128, MPC, H//128]: v[p, m, t] = v_in[m, t*128 + p]
        return np.ascontiguousarray(
            v.reshape(MPC, H // 128, 128).transpose(2, 0, 1))

    in_maps = []
    for c in range(N_CORES):
        s = slice(MPC * c, MPC * (c + 1))
        in_maps.append({
            "xT": xT,
            "w1": np.ascontiguousarray(W1[s]),
            "w2": np.ascontiguousarray(W2[s]),
            "w3": feat_major(W3[s, :, 0]),
            "b1": feat_major(b1[s]),
            "b2": feat_major(b2[s]),
        })

    res = run_bass_kernel_spmd(nc, in_maps, list(range(N_CORES)))
    out = np.concatenate([r["out"] for r in res.results], axis=0)  # [E, B]
    out = out + b3.reshape(E, 1)
    return out.reshape(E, B, 1).astype(np.float32)


# revision 13
# speedup vs baseline: 1.1592x; 1.1592x over previous
"""EnsembleFC (E=16 MLPs, 512->512->512->1, relu) on 8 TRN2 NeuronCores.

Strategy (expert parallel): each core owns E/8 = 2 ensemble members' weights
and computes their [B] output column; x is replicated. All activations stay
in "feature-major" (transposed) layout so no on-device transposes are needed:

    h1^T = relu(W1^T @ x^T + b1)      [H, B]
    h2^T = relu(W2^T @ h1^T + b2)     [H, B]
    out^T = W3^T @ h2^T               [1, B]   (b3 added on host)

Matmuls run in float32r (TRN2 reduced-precision fp32 PE mode, 1 cycle/row —
4x faster than plain fp32, ~20x more accurate than bf16; measured scaled
error ~1.5e-4 per 128-deep contraction).

Raw Bass (no Tile framework): this container's walrus rejects instructions
with more than a couple of sync waits, which Tile's auto-generated drains
exceed. Explicit per-engine programs with standalone waits keep every
instruction at one wait.

Pipeline layout per chunk of 512 batch columns:
  PE:  L1(m0) L1(m1) L2(m0) L2(m1) L3(m0) L3(m1)  -- member interleave hides
       the relu latency between a member's L1 and L2.
  PSUM: each member-layer pair owns 2 banks (mt % 2 rotation); L3 reuses the
       member's first L2 bank at partition 0.
  ACT: relu+bias drains psum into h1/h2 (f32r), copies L3 rows out.
  SP:  weight DMAs (per-tensor sems, split per k-tile), x chunk DMAs
       (per-slot sems -- DMA queue completions are unordered), output stores.
"""
import numpy as np

E, D, H, B = 16, 512, 512, 8192
N_CORES = 8
MPC = E // N_CORES          # members per core
KT = D // 128               # k-tiles per 512 contraction
MT = H // 128               # m-tiles per 512 output dim
CH = 512                    # batch columns per chunk (one psum bank)
NCH = B // CH               # chunks
XBUF = 4                    # x chunk buffering

_CACHE = {}


def _build():
    import concourse.bass as bass
    from concourse import mybir

    f32 = mybir.dt.float32
    f32r = mybir.dt.float32r

    nc = bass.Bass("TRN2", target_bir_lowering=False, debug=False,
                   num_devices=N_CORES)

    xT = nc.dram_tensor("xT", [D, B], f32r, kind="ExternalInput").ap()
    w1 = nc.dram_tensor("w1", [MPC, D, H], f32r, kind="ExternalInput").ap()
    w2 = nc.dram_tensor("w2", [MPC, H, H], f32r, kind="ExternalInput").ap()
    # host-side pre-arranged: w3[p, m, kt], b1/b2[p, m, mt]
    w3 = nc.dram_tensor("w3", [128, MPC, KT], f32r, kind="ExternalInput").ap()
    b1 = nc.dram_tensor("b1", [128, MPC, MT], f32, kind="ExternalInput").ap()
    b2 = nc.dram_tensor("b2", [128, MPC, MT], f32, kind="ExternalInput").ap()
    out = nc.dram_tensor("out", [MPC, B], f32, kind="ExternalOutput").ap()

    w1s = [nc.alloc_sbuf_tensor(f"w1s{m}", [128, KT, H], f32r).ap()
           for m in range(MPC)]
    w2s = [nc.alloc_sbuf_tensor(f"w2s{m}", [128, KT, H], f32r).ap()
           for m in range(MPC)]
    w3s = nc.alloc_sbuf_tensor("w3s", [128, MPC, KT], f32r).ap()
    b1s = nc.alloc_sbuf_tensor("b1s", [128, MPC, MT], f32).ap()
    b2s = nc.alloc_sbuf_tensor("b2s", [128, MPC, MT], f32).ap()
    xs = nc.alloc_sbuf_tensor("xs", [128, XBUF, KT, CH], f32r).ap()
    h1 = nc.alloc_sbuf_tensor("h1", [128, MPC, KT, CH], f32r).ap()
    h2 = nc.alloc_sbuf_tensor("h2", [128, MPC, KT, CH], f32r).ap()
    # per-member output staging, both at partition 0
    osb = [nc.alloc_sbuf_tensor(f"osb{m}", [1, NCH, CH], f32).ap()
           for m in range(MPC)]

    psA = nc.alloc_psum_tensor("psA", [128, 2 * MPC, CH], f32).ap()  # L1
    psB = nc.alloc_psum_tensor("psB", [128, 2 * MPC, CH], f32).ap()  # L2+L3

    xT_r = xT.rearrange("(kt p) b -> p kt b", p=128)

    # --- semaphore tick bookkeeping (absolute counts; 18 groups/chunk) ---
    # PE group order per chunk: L1(m0)x4, L1(m1)x4, L2(m0)x4, L2(m1)x4,
    # L3(m0), L3(m1). ACT mirrors it.
    def mm_l1(c, m, mt):
        return 18 * c + 4 * m + mt + 1

    def mm_l2(c, m, mt):
        return 18 * c + 8 + 4 * m + mt + 1

    def mm_l3(c, m):
        return 18 * c + 16 + m + 1

    act_r1 = mm_l1       # act tick after relu of L1 group (c, m, mt)
    act_r2 = mm_l2
    act_out = mm_l3

    with (
        nc.Block() as block,
        nc.semaphore("mm_sem") as mm_sem,
        nc.semaphore("act_sem") as act_sem,
        nc.semaphore("w3b_sem") as w3b_sem,
        nc.semaphore("d_sem") as d_sem,
    ):
        # per-slot x semaphores: DMA queue completions are unordered across
        # chunks, so a single cumulative counter would be racy
        x_sems = [nc.alloc_semaphore(f"x_sem{s}") for s in range(XBUF)]
        w1_sems = [nc.alloc_semaphore(f"w1_sem{m}") for m in range(MPC)]
        w2_sems = [nc.alloc_semaphore(f"w2_sem{m}") for m in range(MPC)]

        def dma_x(sync, c):
            for kt in range(KT):
                sync.dma_start(
                    out=xs[:, c % XBUF, kt, :],
                    in_=xT_r[:, kt, c * CH:(c + 1) * CH],
                ).then_inc(x_sems[c % XBUF], 16)

        @block.sync
        def _(sync: bass.BassEngine):
            # interleave weight loads with early x chunks, ordered by need
            w1r = [w1[m].rearrange("(kt p) m2 -> p kt m2", p=128)
                   for m in range(MPC)]
            w2r = [w2[m].rearrange("(kt p) m2 -> p kt m2", p=128)
                   for m in range(MPC)]
            for kt in range(KT):
                sync.dma_start(out=w1s[0][:, kt], in_=w1r[0][:, kt]
                               ).then_inc(w1_sems[0], 16)
            sync.dma_start(out=b1s, in_=b1).then_inc(w3b_sem, 16)
            dma_x(sync, 0)
            for kt in range(KT):
                sync.dma_start(out=w1s[1][:, kt], in_=w1r[1][:, kt]
                               ).then_inc(w1_sems[1], 16)
            dma_x(sync, 1)
            for kt in range(KT):
                sync.dma_start(out=w2s[0][:, kt], in_=w2r[0][:, kt]
                               ).then_inc(w2_sems[0], 16)
            dma_x(sync, 2)
            for kt in range(KT):
                sync.dma_start(out=w2s[1][:, kt], in_=w2r[1][:, kt]
                               ).then_inc(w2_sems[1], 16)
            sync.dma_start(out=w3s, in_=w3).then_inc(w3b_sem, 16)
            sync.dma_start(out=b2s, in_=b2).then_inc(w3b_sem, 16)
            dma_x(sync, 3)

            for c in range(XBUF, NCH):
                # x slot free once L1 of chunk c-XBUF fully consumed it
                sync.wait_ge(mm_sem, mm_l1(c - XBUF, MPC - 1, MT - 1))
                dma_x(sync, c)

            out_r = out.rearrange("m (nch ch) -> m nch ch", ch=CH)
            sync.wait_ge(act_sem, act_out(NCH - 1, MPC - 1))
            for m in range(MPC):
                sync.dma_start(out=out_r[m:m + 1], in_=osb[m]).then_inc(d_sem, 16)
            sync.wait_ge(d_sem, 16 * MPC)

        @block.tensor
        def _(tensor: bass.BassEngine):
            for c in range(NCH):
                tensor.wait_ge(x_sems[c % XBUF], 64 * (c // XBUF + 1))
                # L1 both members
                for m in range(MPC):
                    if c == 0:
                        tensor.wait_ge(w1_sems[m], 64)
                    for mt in range(MT):
                        if mt >= 2:           # 2-bank rotation WAR
                            tensor.wait_ge(act_sem, act_r1(c, m, mt - 2))
                        elif c > 0:           # bank last used by c-1, mt+2
                            tensor.wait_ge(act_sem, act_r1(c - 1, m, mt + 2))
                        for kt in range(KT):
                            ins = tensor.matmul(
                                psA[:, 2 * m + mt % 2, :],
                                w1s[m][:, kt, mt * 128:(mt + 1) * 128],
                                xs[:, c % XBUF, kt, :],
                                start=(kt == 0), stop=(kt == KT - 1),
                            )
                        ins.then_inc(mm_sem, 1)
                # L2 both members
                for m in range(MPC):
                    if c == 0:
                        tensor.wait_ge(w2_sems[m], 64)
                    tensor.wait_ge(act_sem, act_r1(c, m, MT - 1))  # h1 ready
                    for mt in range(MT):
                        if mt >= 2:
                            tensor.wait_ge(act_sem, act_r2(c, m, mt - 2))
                        for kt in range(KT):
                            ins = tensor.matmul(
                                psB[:, 2 * m + mt % 2, :],
                                w2s[m][:, kt, mt * 128:(mt + 1) * 128],
                                h1[:, m, kt, :],
                                start=(kt == 0), stop=(kt == KT - 1),
                            )
                        ins.then_inc(mm_sem, 1)
                # L3 both members -> psB bank 2m, partition 0
                for m in range(MPC):
                    if c == 0 and m == 0:
                        tensor.wait_ge(w3b_sem, 48)
                    tensor.wait_ge(act_sem, act_r2(c, m, MT - 1))  # h2 ready
                    for kt in range(KT):
                        ins = tensor.matmul(
                            psB[0:1, 2 * m, :],
                            w3s[:, m, kt:kt + 1],
                            h2[:, m, kt, :],
                            start=(kt == 0), stop=(kt == KT - 1),
                        )
                    ins.then_inc(mm_sem, 1)

        @block.scalar
        def _(scalar: bass.BassEngine):
            Relu = bass.mybir.ActivationFunctionType.Relu
            scalar.wait_ge(w3b_sem, 48)   # b1s/b2s loaded
            for c in range(NCH):
                for m in range(MPC):
                    for mt in range(MT):
                        scalar.wait_ge(mm_sem, mm_l1(c, m, mt))
                        scalar.activation(
                            h1[:, m, mt, :], psA[:, 2 * m + mt % 2, :], Relu,
                            bias=b1s[:, m, mt:mt + 1],
                        ).then_inc(act_sem, 1)
                for m in range(MPC):
                    for mt in range(MT):
                        scalar.wait_ge(mm_sem, mm_l2(c, m, mt))
                        scalar.activation(
                            h2[:, m, mt, :], psB[:, 2 * m + mt % 2, :], Relu,
                            bias=b2s[:, m, mt:mt + 1],
                        ).then_inc(act_sem, 1)
                for m in range(MPC):
                    scalar.wait_ge(mm_sem, mm_l3(c, m))
                    scalar.copy(
                        osb[m][0:1, c, :],
                        psB[0:1, 2 * m, :],
                    ).then_inc(act_sem, 1)

    return nc


def get_nc():
    if "nc" not in _CACHE:
        _CACHE["nc"] = _build()
    return _CACHE["nc"]


def kernel(x, W1, b1, W2, b2, W3, b3):
    from concourse.bass_utils import run_bass_kernel_spmd

    nc = get_nc()
    xT = np.ascontiguousarray(np.asarray(x, dtype=np.float32).T)
    W1 = np.asarray(W1, dtype=np.float32)
    W2 = np.asarray(W2, dtype=np.float32)
    W3 = np.asarray(W3, dtype=np.float32)
    b1 = np.asarray(b1, dtype=np.float32)
    b2 = np.asarray(b2, dtype=np.float32)
    b3 = np.asarray(b3, dtype=np.float32)

    def feat_major(v):
        # [MPC, H] -> [128, MPC, H//128]: v[p, m, t] = v_in[m, t*128 + p]
        return np.ascontiguousarray(
            v.reshape(MPC, H // 128, 128).transpose(2, 0, 1))

    in_maps = []
    for c in range(N_CORES):
        s = slice(MPC * c, MPC * (c + 1))
        in_maps.append({
            "xT": xT,
            "w1": np.ascontiguousarray(W1[s]),
            "w2": np.ascontiguousarray(W2[s]),
            "w3": feat_major(W3[s, :, 0]),
            "b1": feat_major(b1[s]),
            "b2": feat_major(b2[s]),
        })

    res = run_bass_kernel_spmd(nc, in_maps, list(range(N_CORES)))
    out = np.concatenate([r["out"] for r in res.results], axis=0)  # [E, B]
    out = out + b3.reshape(E, 1)
    return out.reshape(E, B, 1).astype(np.float32)


# revision 28
# speedup vs baseline: 1.1670x; 1.0068x over previous
"""EnsembleFC (E=16 MLPs, 512->512->512->1, relu) on 8 TRN2 NeuronCores.

Strategy (expert parallel): each core owns E/8 = 2 ensemble members' weights
and computes their [B] output column; x is replicated. All activations stay
in "feature-major" (transposed) layout so no on-device transposes are needed:

    h1^T = relu(W1^T @ x^T + b1)      [H, B]
    h2^T = relu(W2^T @ h1^T + b2)     [H, B]
    out^T = W3^T @ h2^T               [1, B]   (b3 added on host)

Matmuls run in float32r (TRN2 reduced-precision fp32 PE mode, 1 cycle/row —
4x faster than plain fp32, ~20x more accurate than bf16; measured scaled
error ~1.5e-4 per 128-deep contraction).

Raw Bass (no Tile framework): this container's walrus rejects instructions
with more than a couple of sync waits, which Tile's auto-generated drains
exceed. Explicit per-engine programs with standalone waits keep every
instruction at one wait.

Pipeline layout per chunk of 512 batch columns:
  PE:  L1(m0) L1(m1) L2(m0) L2(m1) L3(m0) L3(m1)  -- member interleave hides
       the relu latency between a member's L1 and L2.
  PSUM: each member-layer pair owns 2 banks (mt % 2 rotation); L3 reuses the
       member's first L2 bank at partition 0.
  ACT: relu+bias drains psum into h1/h2 (f32r), copies L3 rows out.
  SP:  weight DMAs (per-tensor sems, split per k-tile), x chunk DMAs
       (per-slot sems -- DMA queue completions are unordered), output stores.
"""
import numpy as np

E, D, H, B = 16, 512, 512, 8192
N_CORES = 8
MPC = E // N_CORES          # members per core
KT = D // 128               # k-tiles per 512 contraction
MT = H // 128               # m-tiles per 512 output dim
CH = 512                    # batch columns per chunk (one psum bank)
NCH = B // CH               # chunks
XBUF = 4                    # x chunk buffering

_CACHE = {}


def _build():
    import concourse.bass as bass
    from concourse import mybir

    f32 = mybir.dt.float32
    f32r = mybir.dt.float32r

    nc = bass.Bass("TRN2", target_bir_lowering=False, debug=False,
                   num_devices=N_CORES)

    xT = nc.dram_tensor("xT", [D, B], f32r, kind="ExternalInput").ap()
    w1 = nc.dram_tensor("w1", [MPC, D, H], f32r, kind="ExternalInput").ap()
    w2 = nc.dram_tensor("w2", [MPC, H, H], f32r, kind="ExternalInput").ap()
    # host-side pre-arranged: w3[p, m, kt], b1/b2[p, m, mt]
    w3 = nc.dram_tensor("w3", [128, MPC, KT], f32r, kind="ExternalInput").ap()
    b1 = nc.dram_tensor("b1", [128, MPC, MT], f32, kind="ExternalInput").ap()
    b2 = nc.dram_tensor("b2", [128, MPC, MT], f32, kind="ExternalInput").ap()
    out = nc.dram_tensor("out", [MPC, B], f32, kind="ExternalOutput").ap()

    w1s = [nc.alloc_sbuf_tensor(f"w1s{m}", [128, KT, H], f32r).ap()
           for m in range(MPC)]
    w2s = [nc.alloc_sbuf_tensor(f"w2s{m}", [128, KT, H], f32r).ap()
           for m in range(MPC)]
    w3s = nc.alloc_sbuf_tensor("w3s", [128, MPC, KT], f32r).ap()
    b1s = nc.alloc_sbuf_tensor("b1s", [128, MPC, MT], f32).ap()
    b2s = nc.alloc_sbuf_tensor("b2s", [128, MPC, MT], f32).ap()
    xs = nc.alloc_sbuf_tensor("xs", [128, XBUF, KT, CH], f32r).ap()
    h1 = nc.alloc_sbuf_tensor("h1", [128, MPC, KT, CH], f32r).ap()
    h2 = nc.alloc_sbuf_tensor("h2", [128, MPC, KT, CH], f32r).ap()
    # per-member output staging, both at partition 0
    osb = [nc.alloc_sbuf_tensor(f"osb{m}", [1, NCH, CH], f32).ap()
           for m in range(MPC)]

    psA = nc.alloc_psum_tensor("psA", [128, 2 * MPC, CH], f32).ap()  # L1
    psB = nc.alloc_psum_tensor("psB", [128, 2 * MPC, CH], f32).ap()  # L2+L3

    # PE warmup scratch: dummy matmuls during the DMA prologue keep the HAM
    # clock-gate ramp off the critical path
    scr = nc.alloc_sbuf_tensor("scr", [128, 128 + CH], f32r).ap()
    N_WARM = 28

    xT_r = xT.rearrange("(kt p) b -> p kt b", p=128)

    # --- semaphore tick bookkeeping (absolute counts; 18 groups/chunk) ---
    # PE group order per chunk: L1(m0)x4, L1(m1)x4, L2(m0)x4, L2(m1)x4,
    # L3(m0), L3(m1). ACT mirrors it.
    def mm_l1(c, m, mt):
        return 18 * c + 4 * m + mt + 1

    def mm_l2(c, m, mt):
        return 18 * c + 8 + 4 * m + mt + 1

    def mm_l3(c, m):
        return 18 * c + 16 + m + 1

    # ACT does 16 relus per chunk (copies live on DVE)
    def act_r1(c, m, mt):
        return 16 * c + 4 * m + mt + 1

    def act_r2(c, m, mt):
        return 16 * c + 8 + 4 * m + mt + 1

    with (
        nc.Block() as block,
        nc.semaphore("mm_sem") as mm_sem,
        nc.semaphore("act_sem") as act_sem,
        nc.semaphore("b1_sem") as b1_sem,
        nc.semaphore("b2_sem") as b2_sem,
        nc.semaphore("w3_sem") as w3_sem,
        nc.semaphore("d_sem") as d_sem,
    ):
        # per-slot x semaphores: DMA queue completions are unordered across
        # chunks, so a single cumulative counter would be racy
        x_sems = [nc.alloc_semaphore(f"x_sem{s}") for s in range(XBUF)]
        scr_sem = nc.alloc_semaphore("scr_sem")
        cp_sem = nc.alloc_semaphore("cp_sem")
        w1_sems = [nc.alloc_semaphore(f"w1_sem{m}") for m in range(MPC)]
        w2_sems = [nc.alloc_semaphore(f"w2_sem{m}") for m in range(MPC)]

        def dma_x(sync, c):
            for kt in range(KT):
                sync.dma_start(
                    out=xs[:, c % XBUF, kt, :],
                    in_=xT_r[:, kt, c * CH:(c + 1) * CH],
                ).then_inc(x_sems[c % XBUF], 16)

        @block.sync
        def _(sync: bass.BassEngine):
            # interleave weight loads with early x chunks, ordered by need
            w1r = [w1[m].rearrange("(kt p) m2 -> p kt m2", p=128)
                   for m in range(MPC)]
            w2r = [w2[m].rearrange("(kt p) m2 -> p kt m2", p=128)
                   for m in range(MPC)]
            # warmup operands: any finite f32r bits will do
            sync.dma_start(out=scr, in_=xT_r[:, 0, :128 + CH]
                           ).then_inc(scr_sem, 16)
            for kt in range(KT):
                sync.dma_start(out=w1s[0][:, kt], in_=w1r[0][:, kt]
                               ).then_inc(w1_sems[0], 16)
            sync.dma_start(out=b1s, in_=b1).then_inc(b1_sem, 16)
            dma_x(sync, 0)
            for kt in range(KT):
                sync.dma_start(out=w1s[1][:, kt], in_=w1r[1][:, kt]
                               ).then_inc(w1_sems[1], 16)
            sync.dma_start(out=b2s, in_=b2).then_inc(b2_sem, 16)
            sync.dma_start(out=w3s, in_=w3).then_inc(w3_sem, 16)
            dma_x(sync, 1)
            for kt in range(KT):
                sync.dma_start(out=w2s[0][:, kt], in_=w2r[0][:, kt]
                               ).then_inc(w2_sems[0], 16)
            dma_x(sync, 2)
            for kt in range(KT):
                sync.dma_start(out=w2s[1][:, kt], in_=w2r[1][:, kt]
                               ).then_inc(w2_sems[1], 16)
            dma_x(sync, 3)

            out_r = out.rearrange("m (nch ch) -> m nch ch", ch=CH)
            for c in range(XBUF, NCH):
                # x slot free once L1 of chunk c-XBUF fully consumed it
                sync.wait_ge(mm_sem, mm_l1(c - XBUF, MPC - 1, MT - 1))
                dma_x(sync, c)
                # trailing store for chunk c-XBUF (copies long done by now)
                cs = c - XBUF
                sync.wait_ge(cp_sem, MPC * (cs + 1))
                for m in range(MPC):
                    sync.dma_start(out=out_r[m:m + 1, cs],
                                   in_=osb[m][:, cs]).then_inc(d_sem, 16)

            for cs in range(NCH - XBUF, NCH):
                sync.wait_ge(cp_sem, MPC * (cs + 1))
                for m in range(MPC):
                    sync.dma_start(out=out_r[m:m + 1, cs],
                                   in_=osb[m][:, cs]).then_inc(d_sem, 16)
            sync.wait_ge(d_sem, 16 * MPC * NCH)

        @block.vector
        def _(vector: bass.BassEngine):
            # L3 psum -> osb copies live on DVE (otherwise idle): keeps ACT's
            # activation-table pinned to Relu
            for c in range(NCH):
                for m in range(MPC):
                    vector.wait_ge(mm_sem, mm_l3(c, m))
                    vector.tensor_copy(
                        osb[m][0:1, c, :], psB[0:1, 2 * m, :],
                    ).then_inc(cp_sem, 1)

        @block.tensor
        def _(tensor: bass.BassEngine):
            tensor.wait_ge(scr_sem, 16)
            for i in range(N_WARM):
                tensor.matmul(psA[:, 0, :], scr[:, :128], scr[:, 128:],
                              start=True, stop=True, skip_group_check=True)
            for c in range(NCH):
                tensor.wait_ge(x_sems[c % XBUF], 64 * (c // XBUF + 1))
                # L1 both members
                for m in range(MPC):
                    if c == 0:
                        tensor.wait_ge(w1_sems[m], 64)
                    for mt in range(MT):
                        if mt >= 2:           # 2-bank rotation WAR
                            tensor.wait_ge(act_sem, act_r1(c, m, mt - 2))
                        elif c > 0:           # bank last used by c-1, mt+2
                            tensor.wait_ge(act_sem, act_r1(c - 1, m, mt + 2))
                        for kt in range(KT):
                            ins = tensor.matmul(
                                psA[:, 2 * m + mt % 2, :],
                                w1s[m][:, kt, mt * 128:(mt + 1) * 128],
                                xs[:, c % XBUF, kt, :],
                                start=(kt == 0), stop=(kt == KT - 1),
                            )
                        ins.then_inc(mm_sem, 1)
                # L2 both members
                for m in range(MPC):
                    if c == 0:
                        tensor.wait_ge(w2_sems[m], 64)
                    if c > 0:
                        # psB bank 2m holds chunk c-1's L3 row until DVE
                        # copies it out
                        tensor.wait_ge(cp_sem, 2 * (c - 1) + m + 1)
                    tensor.wait_ge(act_sem, act_r1(c, m, MT - 1))  # h1 ready
                    for mt in range(MT):
                        if mt >= 2:
                            tensor.wait_ge(act_sem, act_r2(c, m, mt - 2))
                        for kt in range(KT):
                            ins = tensor.matmul(
                                psB[:, 2 * m + mt % 2, :],
                                w2s[m][:, kt, mt * 128:(mt + 1) * 128],
                                h1[:, m, kt, :],
                                start=(kt == 0), stop=(kt == KT - 1),
                            )
                        ins.then_inc(mm_sem, 1)
                # L3 both members -> psB bank 2m, partition 0
                for m in range(MPC):
                    if c == 0 and m == 0:
                        tensor.wait_ge(w3_sem, 16)
                    tensor.wait_ge(act_sem, act_r2(c, m, MT - 1))  # h2 ready
                    for kt in range(KT):
                        ins = tensor.matmul(
                            psB[0:1, 2 * m, :],
                            w3s[:, m, kt:kt + 1],
                            h2[:, m, kt, :],
                            start=(kt == 0), stop=(kt == KT - 1),
                        )
                    ins.then_inc(mm_sem, 1)

        @block.scalar
        def _(scalar: bass.BassEngine):
            Relu = bass.mybir.ActivationFunctionType.Relu
            scalar.wait_ge(b1_sem, 16)
            scalar.wait_ge(b2_sem, 16)
            for c in range(NCH):
                for m in range(MPC):
                    for mt in range(MT):
                        scalar.wait_ge(mm_sem, mm_l1(c, m, mt))
                        scalar.activation(
                            h1[:, m, mt, :], psA[:, 2 * m + mt % 2, :], Relu,
                            bias=b1s[:, m, mt:mt + 1],
                        ).then_inc(act_sem, 1)
                for m in range(MPC):
                    for mt in range(MT):
                        scalar.wait_ge(mm_sem, mm_l2(c, m, mt))
                        scalar.activation(
                            h2[:, m, mt, :], psB[:, 2 * m + mt % 2, :], Relu,
                            bias=b2s[:, m, mt:mt + 1],
                        ).then_inc(act_sem, 1)

    return nc


def get_nc():
    if "nc" not in _CACHE:
        _CACHE["nc"] = _build()
    return _CACHE["nc"]


def kernel(x, W1, b1, W2, b2, W3, b3):
    from concourse.bass_utils import run_bass_kernel_spmd

    nc = get_nc()
    xT = np.ascontiguousarray(np.asarray(x, dtype=np.float32).T)
    W1 = np.asarray(W1, dtype=np.float32)
    W2 = np.asarray(W2, dtype=np.float32)
    W3 = np.asarray(W3, dtype=np.float32)
    b1 = np.asarray(b1, dtype=np.float32)
    b2 = np.asarray(b2, dtype=np.float32)
    b3 = np.asarray(b3, dtype=np.float32)

    def feat_major(v):
        # [MPC, H] -> [128, MPC, H//128]: v[p, m, t] = v_in[m, t*128 + p]
        return np.ascontiguousarray(
            v.reshape(MPC, H // 128, 128).transpose(2, 0, 1))

    in_maps = []
    for c in range(N_CORES):
        s = slice(MPC * c, MPC * (c + 1))
        in_maps.append({
            "xT": xT,
            "w1": np.ascontiguousarray(W1[s]),
            "w2": np.ascontiguousarray(W2[s]),
            "w3": feat_major(W3[s, :, 0]),
            "b1": feat_major(b1[s]),
            "b2": feat_major(b2[s]),
        })

    res = run_bass_kernel_spmd(nc, in_maps, list(range(N_CORES)))
    out = np.concatenate([r["out"] for r in res.results], axis=0)  # [E, B]
    out = out + b3.reshape(E, 1)
    return out.reshape(E, B, 1).astype(np.float32)


# revision 30
# speedup vs baseline: 1.2185x; 1.0441x over previous
"""EnsembleFC (E=16 MLPs, 512->512->512->1, relu) on 8 TRN2 NeuronCores.

Strategy (expert parallel): each core owns E/8 = 2 ensemble members' weights
and computes their [B] output column; x is replicated. All activations stay
in "feature-major" (transposed) layout so no on-device transposes are needed:

    h1^T = relu(W1^T @ x^T + b1)      [H, B]
    h2^T = relu(W2^T @ h1^T + b2)     [H, B]
    out^T = W3^T @ h2^T               [1, B]   (b3 added on host)

Matmuls run in float32r (TRN2 reduced-precision fp32 PE mode, 1 cycle/row —
4x faster than plain fp32, ~20x more accurate than bf16; measured scaled
error ~1.5e-4 per 128-deep contraction).

Raw Bass (no Tile framework): this container's walrus rejects instructions
with more than a couple of sync waits, which Tile's auto-generated drains
exceed. Explicit per-engine programs with standalone waits keep every
instruction at one wait.

Pipeline layout per chunk of 512 batch columns:
  PE:  L1(m0) L1(m1) L2(m0) L2(m1) L3(m0) L3(m1)  -- member interleave hides
       the relu latency between a member's L1 and L2.
  PSUM: each member-layer pair owns 2 banks (mt % 2 rotation); L3 reuses the
       member's first L2 bank at partition 0.
  ACT: relu+bias drains psum into h1/h2 (f32r), copies L3 rows out.
  SP:  weight DMAs (per-tensor sems, split per k-tile), x chunk DMAs
       (per-slot sems -- DMA queue completions are unordered), output stores.
"""
import numpy as np

E, D, H, B = 16, 512, 512, 8192
N_CORES = 8
MPC = E // N_CORES          # members per core
KT = D // 128               # k-tiles per 512 contraction
MT = H // 128               # m-tiles per 512 output dim
CH = 512                    # batch columns per chunk (one psum bank)
NCH = B // CH               # chunks
XBUF = 4                    # x chunk buffering

_CACHE = {}


def _build():
    import concourse.bass as bass
    from concourse import mybir

    f32 = mybir.dt.float32
    f32r = mybir.dt.float32r

    nc = bass.Bass("TRN2", target_bir_lowering=False, debug=False,
                   num_devices=N_CORES)

    xT = nc.dram_tensor("xT", [D, B], f32r, kind="ExternalInput").ap()
    w1 = nc.dram_tensor("w1", [MPC, D, H], f32r, kind="ExternalInput").ap()
    w2 = nc.dram_tensor("w2", [MPC, H, H], f32r, kind="ExternalInput").ap()
    # host-side pre-arranged: w3[p, m, kt], b1/b2[p, m, mt]
    w3 = nc.dram_tensor("w3", [128, MPC, KT], f32r, kind="ExternalInput").ap()
    b1 = nc.dram_tensor("b1", [128, MPC, MT], f32, kind="ExternalInput").ap()
    b2 = nc.dram_tensor("b2", [128, MPC, MT], f32, kind="ExternalInput").ap()
    out = nc.dram_tensor("out", [MPC, B], f32, kind="ExternalOutput").ap()

    w1s = [nc.alloc_sbuf_tensor(f"w1s{m}", [128, KT, H], f32r).ap()
           for m in range(MPC)]
    w2s = [nc.alloc_sbuf_tensor(f"w2s{m}", [128, KT, H], f32r).ap()
           for m in range(MPC)]
    w3s = nc.alloc_sbuf_tensor("w3s", [128, MPC, KT], f32r).ap()
    b1s = nc.alloc_sbuf_tensor("b1s", [128, MPC, MT], f32).ap()
    b2s = nc.alloc_sbuf_tensor("b2s", [128, MPC, MT], f32).ap()
    xs = nc.alloc_sbuf_tensor("xs", [128, XBUF, KT, CH], f32r).ap()
    h1 = nc.alloc_sbuf_tensor("h1", [128, MPC, KT, CH], f32r).ap()
    h2 = nc.alloc_sbuf_tensor("h2", [128, MPC, KT, CH], f32r).ap()
    # per-member output staging, both at partition 0
    osb = [nc.alloc_sbuf_tensor(f"osb{m}", [1, NCH, CH], f32).ap()
           for m in range(MPC)]

    psA = nc.alloc_psum_tensor("psA", [128, 2 * MPC, CH], f32).ap()  # L1
    psB = nc.alloc_psum_tensor("psB", [128, 2 * MPC, CH], f32).ap()  # L2+L3

    # PE warmup scratch: dummy matmuls during the DMA prologue keep the HAM
    # clock-gate ramp off the critical path
    scr = nc.alloc_sbuf_tensor("scr", [128, 128 + CH], f32r).ap()
    N_WARM = 28

    xT_r = xT.rearrange("(kt p) b -> p kt b", p=128)

    # --- semaphore tick bookkeeping (absolute counts; 18 groups/chunk) ---
    # PE group order per chunk: L1(m0)x4, L1(m1)x4, L2(m0)x4, L2(m1)x4,
    # L3(m0), L3(m1). ACT mirrors it.
    def mm_l1(c, m, mt):
        return 18 * c + 4 * m + mt + 1

    def mm_l2(c, m, mt):
        return 18 * c + 8 + 4 * m + mt + 1

    def mm_l3(c, m):
        return 18 * c + 16 + m + 1

    # ACT does 16 relus per chunk (copies live on DVE)
    def act_r1(c, m, mt):
        return 16 * c + 4 * m + mt + 1

    def act_r2(c, m, mt):
        return 16 * c + 8 + 4 * m + mt + 1

    with (
        nc.Block() as block,
        nc.semaphore("mm_sem") as mm_sem,
        nc.semaphore("act_sem") as act_sem,
        nc.semaphore("b1_sem") as b1_sem,
        nc.semaphore("b2_sem") as b2_sem,
        nc.semaphore("w3_sem") as w3_sem,
        nc.semaphore("d_sem") as d_sem,
    ):
        # per-slot x semaphores: DMA queue completions are unordered across
        # chunks, so a single cumulative counter would be racy
        x_sems = [nc.alloc_semaphore(f"x_sem{s}") for s in range(XBUF)]
        scr_sem = nc.alloc_semaphore("scr_sem")
        cp_sem = nc.alloc_semaphore("cp_sem")
        w1_sems = [nc.alloc_semaphore(f"w1_sem{m}") for m in range(MPC)]
        w2_sems = [nc.alloc_semaphore(f"w2_sem{m}") for m in range(MPC)]

        def dma_x(sync, c):
            for kt in range(KT):
                sync.dma_start(
                    out=xs[:, c % XBUF, kt, :],
                    in_=xT_r[:, kt, c * CH:(c + 1) * CH],
                ).then_inc(x_sems[c % XBUF], 16)

        @block.sync
        def _(sync: bass.BassEngine):
            # interleave weight loads with early x chunks, ordered by need
            w1r = [w1[m].rearrange("(kt p) m2 -> p kt m2", p=128)
                   for m in range(MPC)]
            w2r = [w2[m].rearrange("(kt p) m2 -> p kt m2", p=128)
                   for m in range(MPC)]

            for kt in range(KT):
                sync.dma_start(out=w1s[0][:, kt], in_=w1r[0][:, kt]
                               ).then_inc(w1_sems[0], 16)
            sync.dma_start(out=b1s, in_=b1).then_inc(b1_sem, 16)
            dma_x(sync, 0)
            for kt in range(KT):
                sync.dma_start(out=w1s[1][:, kt], in_=w1r[1][:, kt]
                               ).then_inc(w1_sems[1], 16)
            sync.dma_start(out=b2s, in_=b2).then_inc(b2_sem, 16)
            sync.dma_start(out=w3s, in_=w3).then_inc(w3_sem, 16)
            dma_x(sync, 1)
            for kt in range(KT):
                sync.dma_start(out=w2s[0][:, kt], in_=w2r[0][:, kt]
                               ).then_inc(w2_sems[0], 16)
            dma_x(sync, 2)
            for kt in range(KT):
                sync.dma_start(out=w2s[1][:, kt], in_=w2r[1][:, kt]
                               ).then_inc(w2_sems[1], 16)
            dma_x(sync, 3)

            out_r = out.rearrange("m (nch ch) -> m nch ch", ch=CH)
            for c in range(XBUF, NCH):
                # x slot free once L1 of chunk c-XBUF fully consumed it
                sync.wait_ge(mm_sem, mm_l1(c - XBUF, MPC - 1, MT - 1))
                dma_x(sync, c)
                # trailing store for chunk c-XBUF (copies long done by now)
                cs = c - XBUF
                sync.wait_ge(cp_sem, MPC * (cs + 1))
                for m in range(MPC):
                    sync.dma_start(out=out_r[m:m + 1, cs],
                                   in_=osb[m][:, cs]).then_inc(d_sem, 16)

            for cs in range(NCH - XBUF, NCH):
                sync.wait_ge(cp_sem, MPC * (cs + 1))
                for m in range(MPC):
                    sync.dma_start(out=out_r[m:m + 1, cs],
                                   in_=osb[m][:, cs]).then_inc(d_sem, 16)
            sync.wait_ge(d_sem, 16 * MPC * NCH)

        @block.vector
        def _(vector: bass.BassEngine):
            # L3 psum -> osb copies live on DVE (otherwise idle): keeps ACT's
            # activation-table pinned to Relu
            for c in range(NCH):
                for m in range(MPC):
                    vector.wait_ge(mm_sem, mm_l3(c, m))
                    vector.tensor_copy(
                        osb[m][0:1, c, :], psB[0:1, 2 * m, :],
                    ).then_inc(cp_sem, 1)

        @block.tensor
        def _(tensor: bass.BassEngine):
            # warmup on uninitialized scratch: values are irrelevant, the psum
            # is overwritten (start=True) before any reader
            for i in range(N_WARM):
                tensor.matmul(psA[:, 0, :], scr[:, :128], scr[:, 128:],
                              start=True, stop=True, skip_group_check=True)
            for c in range(NCH):
                tensor.wait_ge(x_sems[c % XBUF], 64 * (c // XBUF + 1))
                # L1 both members
                for m in range(MPC):
                    if c == 0:
                        tensor.wait_ge(w1_sems[m], 64)
                    for mt in range(MT):
                        if mt >= 2:           # 2-bank rotation WAR
                            tensor.wait_ge(act_sem, act_r1(c, m, mt - 2))
                        elif c > 0:           # bank last used by c-1, mt+2
                            tensor.wait_ge(act_sem, act_r1(c - 1, m, mt + 2))
                        for kt in range(KT):
                            ins = tensor.matmul(
                                psA[:, 2 * m + mt % 2, :],
                                w1s[m][:, kt, mt * 128:(mt + 1) * 128],
                                xs[:, c % XBUF, kt, :],
                                start=(kt == 0), stop=(kt == KT - 1),
                            )
                        ins.then_inc(mm_sem, 1)
                # L2 both members
                for m in range(MPC):
                    if c == 0:
                        tensor.wait_ge(w2_sems[m], 64)
                    if c > 0:
                        # psB bank 2m holds chunk c-1's L3 row until DVE
                        # copies it out
                        tensor.wait_ge(cp_sem, 2 * (c - 1) + m + 1)
                    tensor.wait_ge(act_sem, act_r1(c, m, MT - 1))  # h1 ready
                    for mt in range(MT):
                        if mt >= 2:
                            tensor.wait_ge(act_sem, act_r2(c, m, mt - 2))
                        for kt in range(KT):
                            ins = tensor.matmul(
                                psB[:, 2 * m + mt % 2, :],
                                w2s[m][:, kt, mt * 128:(mt + 1) * 128],
                                h1[:, m, kt, :],
                                start=(kt == 0), stop=(kt == KT - 1),
                            )
                        ins.then_inc(mm_sem, 1)
                # L3 both members -> psB bank 2m, partition 0
                for m in range(MPC):
                    if c == 0 and m == 0:
                        tensor.wait_ge(w3_sem, 16)
                    tensor.wait_ge(act_sem, act_r2(c, m, MT - 1))  # h2 ready
                    for kt in range(KT):
                        ins = tensor.matmul(
                            psB[0:1, 2 * m, :],
                            w3s[:, m, kt:kt + 1],
                            h2[:, m, kt, :],
                            start=(kt == 0), stop=(kt == KT - 1),
                        )
                    ins.then_inc(mm_sem, 1)

        @block.scalar
        def _(scalar: bass.BassEngine):
            Relu = bass.mybir.ActivationFunctionType.Relu
            scalar.wait_ge(b1_sem, 16)
            scalar.wait_ge(b2_sem, 16)
            for c in range(NCH):
                for m in range(MPC):
                    for mt in range(MT):
                        scalar.wait_ge(mm_sem, mm_l1(c, m, mt))
                        scalar.activation(
                            h1[:, m, mt, :], psA[:, 2 * m + mt % 2, :], Relu,
                            bias=b1s[:, m, mt:mt + 1],
                        ).then_inc(act_sem, 1)
                for m in range(MPC):
                    for mt in range(MT):
                        scalar.wait_ge(mm_sem, mm_l2(c, m, mt))
                        scalar.activation(
                            h2[:, m, mt, :], psB[:, 2 * m + mt % 2, :], Relu,
                            bias=b2s[:, m, mt:mt + 1],
                        ).then_inc(act_sem, 1)

    return nc


def get_nc():
    if "nc" not in _CACHE:
        _CACHE["nc"] = _build()
    return _CACHE["nc"]


def kernel(x, W1, b1, W2, b2, W3, b3):
    from concourse.bass_utils import run_bass_kernel_spmd

    nc = get_nc()
    xT = np.ascontiguousarray(np.asarray(x, dtype=np.float32).T)
    W1 = np.asarray(W1, dtype=np.float32)
    W2 = np.asarray(W2, dtype=np.float32)
    W3 = np.asarray(W3, dtype=np.float32)
    b1 = np.asarray(b1, dtype=np.float32)
    b2 = np.asarray(b2, dtype=np.float32)
    b3 = np.asarray(b3, dtype=np.float32)

    def feat_major(v):
        # [MPC, H] -> [128, MPC, H//128]: v[p, m, t] = v_in[m, t*128 + p]
        return np.ascontiguousarray(
            v.reshape(MPC, H // 128, 128).transpose(2, 0, 1))

    in_maps = []
    for c in range(N_CORES):
        s = slice(MPC * c, MPC * (c + 1))
        in_maps.append({
            "xT": xT,
            "w1": np.ascontiguousarray(W1[s]),
            "w2": np.ascontiguousarray(W2[s]),
            "w3": feat_major(W3[s, :, 0]),
            "b1": feat_major(b1[s]),
            "b2": feat_major(b2[s]),
        })

    res = run_bass_kernel_spmd(nc, in_maps, list(range(N_CORES)))
    out = np.concatenate([r["out"] for r in res.results], axis=0)  # [E, B]
    out = out + b3.reshape(E, 1)
    return out.reshape(E, B, 1).astype(np.float32)
